# revision 6
# baseline (speedup 1.0000x reference)
"""Trainium2 Bass kernel for a dense transformer block (pre-LN, causal MHA + FFN).

Shapes (hardcoded): x [1024, 64, 384] fp32, 6 heads x 64, FFN hidden 1536.
Strategy: data-parallel over batch across 8 NeuronCores (128 seqs/core), no
collectives. Per core, one fused loop over segments of 8 token tiles
(16 sequences): LN1 -> QKV -> causal attention -> proj+residual -> LN2 ->
FFN+residual. Dense matmuls (QKV/proj/FFN) run in fp8e4 with DoubleRow
perf mode (K=256 per instruction, ~1.6-2x over bf16 measured); attention
(scores/AV) stays bf16; LN/softmax/residual math in fp32. LN affine params
are folded into the weights host-side; bias terms are handled generally
(K=1 rank-1 matmuls / activation bias), emitted only when nonzero.

Layout notes (contraction must sit on SBUF partitions for both operands):
 - weights are stored fp8 as a DoubleRow pair (C-tiles {0,1} side by side:
   [128, 2, M]) plus a plain tail tile (C-tile 2); contraction C=384 is
   covered by one DoubleRow matmul (K=256) + one single fp8 matmul (K=128).
   W2's K=1536 is 6 DoubleRow pairs, no tail.
 - xnF: LN1 output transposed to feature-major (fp8) via bf16 PE transposes
   + casting PSUM->SBUF copies; serves as moving operand for q/k and
   stationary operand for v.
 - attention computes S^T = k @ q^T directly (scores transposed, [s, t]) so
   the softmax matrix is already stationary-ready for the AV matmul; the
   softmax denominator comes free as an extra ones-column in the v operand.
 - softmax skips the max-subtraction: scores are O(1) by construction
   (LN'd activations times 0.02-scale weights), exp is safe in fp32.
Engine balance: PE ~42us/segment is the roofline; exp/bias-evac/relu/copies
split across ACT+DVE (~33/36us); LN apply and causal mask on GPSIMD (~21us).
"""

import os
import sys

import numpy as np

for _p in ("/opt/trn_rl_repo", os.path.expanduser("~/.axon_site/_ro/trn_rl_repo")):
    if os.path.isdir(_p) and _p not in sys.path:
        sys.path.insert(0, _p)

import ml_dtypes  # noqa: E402
import concourse.hw_specs as _hw_specs  # noqa: E402
import concourse.bacc as bacc  # noqa: E402
import concourse.tile as tile  # noqa: E402
from concourse import mybir  # noqa: E402
from concourse.bass_utils import run_bass_kernel_spmd  # noqa: E402

# Pin every activation function this kernel uses (Exp/Ln/Identity/Copy/Relu)
# to the one act table that contains them all (natural_log_exp_and_others,
# set id 6). The default per-instruction table choice ping-pongs between
# tables, costing a ~1.3us table reload per switch on the ACT engine. Table
# ids keep their true act_info.json positions, so walrus emits correct
# act.json entries.
_ACT_PIN = {mybir.ActivationFunctionType.Exp, mybir.ActivationFunctionType.Ln,
            mybir.ActivationFunctionType.Identity,
            mybir.ActivationFunctionType.Copy,
            mybir.ActivationFunctionType.Relu}
_orig_get_tables = _hw_specs.get_activation_tables


def _pinned_tables(arch):
    out = {}
    for name, fns in _orig_get_tables(arch).items():
        out[name] = fns if name == "natural_log_exp_and_others" \
            else fns - _ACT_PIN
    return out


_hw_specs.get_activation_tables = _pinned_tables
bacc.get_activation_tables = _pinned_tables

BF16 = mybir.dt.bfloat16
FP8 = mybir.dt.float8e4
F32 = mybir.dt.float32
ACTF = mybir.ActivationFunctionType
ALU = mybir.AluOpType
DR = mybir.MatmulPerfMode.DoubleRow

N_CORES = 8
B_FULL, T, C, H, D = 1024, 64, 384, 6, 64
J = 4 * C                       # 1536
B_LOC = B_FULL // N_CORES       # 128 sequences per core
NTOK = B_LOC * T                # 8192 tokens per core
P = 128
NT = NTOK // P                  # 64 token tiles (each tile = one pair of seqs)
SEG = 8                         # token tiles per fused segment
KC = C // P                     # 3 contraction chunks over C
JC = J // P                     # 12 chunks over FFN hidden
EPS = 1e-5
SCALE = D ** -0.5

_CACHE = {}
last_exec_time_ns = None


def _build(has_bv, has_bo, has_b2, nt=NT, loop_n=1):
    assert nt % SEG == 0 and (SEG * P) % 512 == 0
    nc = bacc.Bacc("TRN2", target_bir_lowering=False, debug=False)
    ntok = nt * P
    nseg = nt // SEG
    SW = SEG * P                # tokens per segment (1024)

    x_d = nc.dram_tensor("x", [ntok, C], F32, kind="ExternalInput").ap()
    wqp_d = nc.dram_tensor("wqp", [P, 2 * C], FP8, kind="ExternalInput").ap()
    wqt_d = nc.dram_tensor("wqt", [P, C], FP8, kind="ExternalInput").ap()
    wkp_d = nc.dram_tensor("wkp", [P, 2 * C], FP8, kind="ExternalInput").ap()
    wkt_d = nc.dram_tensor("wkt", [P, C], FP8, kind="ExternalInput").ap()
    wvp_d = nc.dram_tensor("wvp", [P, 2 * C], FP8, kind="ExternalInput").ap()
    wvt_d = nc.dram_tensor("wvt", [P, C], FP8, kind="ExternalInput").ap()
    wop_d = nc.dram_tensor("wop", [P, 2 * C], FP8, kind="ExternalInput").ap()
    wot_d = nc.dram_tensor("wot", [P, C], FP8, kind="ExternalInput").ap()
    w1p_d = nc.dram_tensor("w1p", [P, 2 * J], FP8, kind="ExternalInput").ap()
    w1t_d = nc.dram_tensor("w1t", [P, J], FP8, kind="ExternalInput").ap()
    w2_d = nc.dram_tensor("w2", [P, JC * C], FP8, kind="ExternalInput").ap()
    bq_d = nc.dram_tensor("bq", [P, KC], F32, kind="ExternalInput").ap()
    bk_d = nc.dram_tensor("bk", [P, KC], F32, kind="ExternalInput").ap()
    bh_d = nc.dram_tensor("bh", [P, JC], F32, kind="ExternalInput").ap()
    bv_d = nc.dram_tensor("bv", [1, C], BF16, kind="ExternalInput").ap()
    bo_d = nc.dram_tensor("bo_r", [1, C], BF16, kind="ExternalInput").ap()
    b2_d = nc.dram_tensor("b2_r", [1, C], BF16, kind="ExternalInput").ap()
    id_d = nc.dram_tensor("ident", [P, P], BF16, kind="ExternalInput").ap()
    mk_d = nc.dram_tensor("maskt", [P, H * P], BF16, kind="ExternalInput").ap()
    out_d = nc.dram_tensor("out", [ntok, C], F32, kind="ExternalOutput").ap()

    with tile.TileContext(nc) as tc:
        with tc.tile_pool(name="singles", bufs=1) as sg, \
             tc.tile_pool(name="seg", bufs=2) as sgp, \
             tc.tile_pool(name="work", bufs=5) as wk, \
             tc.tile_pool(name="psum", bufs=1, space="PSUM") as ps:

            # ---- resident weights / constants (fp8 pair+tail layout) ----
            wqp = sg.tile([P, 2 * C], FP8, name="wqp")
            wqt = sg.tile([P, C], FP8, name="wqt")
            wkp = sg.tile([P, 2 * C], FP8, name="wkp")
            wkt = sg.tile([P, C], FP8, name="wkt")
            wvp = sg.tile([P, 2 * C], FP8, name="wvp")
            wvt = sg.tile([P, C], FP8, name="wvt")
            wop = sg.tile([P, 2 * C], FP8, name="wop")
            wot = sg.tile([P, C], FP8, name="wot")
            w1p = sg.tile([P, 2 * J], FP8, name="w1p")
            w1t = sg.tile([P, J], FP8, name="w1t")
            w28 = sg.tile([P, JC * C], FP8, name="w28")
            nc.gpsimd.dma_start(out=wqp, in_=wqp_d)
            nc.gpsimd.dma_start(out=wqt, in_=wqt_d)
            nc.gpsimd.dma_start(out=wkp, in_=wkp_d)
            nc.gpsimd.dma_start(out=wkt, in_=wkt_d)
            nc.gpsimd.dma_start(out=wvp, in_=wvp_d)
            nc.gpsimd.dma_start(out=wvt, in_=wvt_d)
            nc.gpsimd.dma_start(out=wop, in_=wop_d)
            nc.gpsimd.dma_start(out=wot, in_=wot_d)
            nc.gpsimd.dma_start(out=w1p, in_=w1p_d)
            nc.gpsimd.dma_start(out=w1t, in_=w1t_d)
            nc.gpsimd.dma_start(out=w28, in_=w2_d)
            # 3D views for DoubleRow APs
            wqp_v = wqp.rearrange("p (a c) -> p a c", a=2)
            wkp_v = wkp.rearrange("p (a c) -> p a c", a=2)
            wvp_v = wvp.rearrange("p (a c) -> p a c", a=2)
            wop_v = wop.rearrange("p (a c) -> p a c", a=2)
            w1p_v = w1p.rearrange("p (a j) -> p a j", a=2)
            w28_v = w28.rearrange("p (j c) -> p j c", j=JC)
            bq_sb = sg.tile([P, KC], F32)
            bk_sb = sg.tile([P, KC], F32)
            bh_sb = sg.tile([P, JC], F32)
            ident = sg.tile([P, P], BF16)
            maskt = sg.tile([P, H * P], BF16)
            nc.sync.dma_start(out=ident, in_=id_d)
            nc.scalar.dma_start(out=bq_sb, in_=bq_d)
            nc.scalar.dma_start(out=bk_sb, in_=bk_d)
            nc.scalar.dma_start(out=bh_sb, in_=bh_d)
            nc.scalar.dma_start(out=maskt, in_=mk_d)
            eps_sb = sg.tile([P, 1], F32)
            nc.vector.memset(eps_sb, EPS)
            ones1 = sg.tile([1, P], BF16)
            nc.vector.memset(ones1, 1.0)
            bv_sb = sg.tile([1, C], BF16)
            bo_sb = sg.tile([1, C], BF16)
            b2_sb = sg.tile([1, C], BF16)
            if has_bv:
                nc.sync.dma_start(out=bv_sb, in_=bv_d)
            if has_bo:
                nc.sync.dma_start(out=bo_sb, in_=bo_d)
            if has_b2:
                nc.sync.dma_start(out=b2_sb, in_=b2_d)

            def bass_strided(dstF, t):
                # [P, KC, 128] view of dstF hitting columns k*SW + t*128
                return dstF.rearrange("p (k w) -> p k w", k=KC)[
                    :, :, t * P:(t + 1) * P]

            def _copy(idx, out, in_):
                if idx % 2 == 0:
                    nc.scalar.copy(out=out, in_=in_)
                else:
                    nc.vector.tensor_copy(out=out, in_=in_)

            def ln_chain(src_f32, xn0_tag):
                """layernorm (no affine) of a [128, C] fp32 tile -> bf16 ->
                later transposed into feature-major fp8.
                rstd = exp(-0.5*ln(var+eps)) keeps every ACT op in the one
                natural_log_exp_and_others table (no table reloads). The
                elementwise apply runs on GPSIMD (only engine with slack)."""
                stats = wk.tile([P, 6], F32, tag="lnstats")
                mv = wk.tile([P, 2], F32, tag="lnmv")
                nc.vector.bn_stats(out=stats, in_=src_f32)
                nc.vector.bn_aggr(out=mv, in_=stats)
                lnv = wk.tile([P, 1], F32, tag="lnlnv")
                nc.scalar.activation(out=lnv, in_=mv[:, 1:2], func=ACTF.Ln,
                                     bias=eps_sb, scale=1.0)
                rstd = wk.tile([P, 1], F32, tag="lnrstd")
                nc.scalar.activation(out=rstd, in_=lnv, func=ACTF.Exp,
                                     bias=0.0, scale=-0.5)
                xn0 = wk.tile([P, C], BF16, tag=xn0_tag, bufs=4,
                              name="xn0")
                nc.gpsimd.tensor_scalar(out=xn0, in0=src_f32,
                                        scalar1=mv[:, 0:1], scalar2=rstd,
                                        op0=ALU.subtract, op1=ALU.mult)
                return xn0

            def tp_to_F(t, xn0, dstF, eng_off=0):
                tp = ps.tile([P, C], BF16, tag="big", bufs=2, name="tp")
                for k in range(KC):
                    nc.tensor.transpose(tp[:, k * P:(k + 1) * P],
                                        xn0[:, k * P:(k + 1) * P], ident)
                # one strided copy: chunk k lands at column k*SW + t*128
                _copy(t + eng_off, bass_strided(dstF, t),
                      tp.rearrange("p (k c) -> p k c", c=P))

            ng = SW // 512

            def new_state(s):
                st_ = {"i0": s * SEG}
                st_["xnF"] = sgp.tile([P, KC * SW], FP8, tag="xnF",
                                      name="xnF")
                st_["qF"] = [sgp.tile([P, SW], BF16, tag=f"qF{m}",
                                      name=f"qF{m}") for m in range(KC)]
                st_["kF"] = [sgp.tile([P, SW], BF16, tag=f"kF{m}",
                                      name=f"kF{m}") for m in range(KC)]
                st_["vaug"] = sgp.tile([P, SEG, H, D + 1], BF16, tag="vaug",
                                       name="vaug")
                st_["attn"] = sgp.tile([P, SEG * C], BF16, tag="attn",
                                       name="attn")
                st_["xn2F"] = sgp.tile([P, KC * SW], FP8, tag="xn2F",
                                       name="xn2F")
                st_["x2"] = sgp.tile([P, SEG, C], F32, tag="x2", name="x2")
                return st_

            def emit_A_tile(st_, t):
                emit_A_ln(st_, t)
                emit_A_tp(st_, t)

            def emit_A_ln(st_, t):
                xt = wk.tile([P, C], F32, tag="xa")
                i0 = st_["i0"]
                nc.sync.dma_start(
                    out=xt, in_=x_d[(i0 + t) * P:(i0 + t + 1) * P, :])
                st_[f"xn0_{t}"] = ln_chain(xt, "lnxn0A")

            def emit_A_tp(st_, t):
                tp_to_F(t, st_.pop(f"xn0_{t}"), st_["xnF"])

            def emit_B(st_):
                xnF, qF, kF = st_["xnF"], st_["qF"], st_["kF"]
                xnF_v = xnF.rearrange("p (k w) -> p k w", k=KC)
                for m in range(KC):
                    for g in range(ng):
                        for wp_v, wt, dstF, bias in (
                                (wqp_v, wqt, qF, bq_sb),
                                (wkp_v, wkt, kF, bk_sb)):
                            pqk = ps.tile([P, 512], F32, tag="st", bufs=2)
                            nc.tensor.matmul(
                                pqk, wp_v[:, :, m * P:(m + 1) * P],
                                xnF_v[:, 0:2, g * 512:(g + 1) * 512],
                                start=True, stop=False, perf_mode=DR)
                            nc.tensor.matmul(
                                pqk, wt[:, m * P:(m + 1) * P],
                                xnF_v[:, 2, g * 512:(g + 1) * 512],
                                start=False, stop=True)
                            nc.scalar.activation(
                                out=dstF[m][:, g * 512:(g + 1) * 512],
                                in_=pqk, func=ACTF.Identity,
                                bias=bias[:, m:m + 1], scale=1.0)
                # v projection (xnF stationary -> T-layout, plus ones column)
                vaug = st_["vaug"]
                nc.vector.memset(vaug[:, :, :, D:D + 1], 1.0)
                for t in range(SEG):
                    pvf = ps.tile([P, 512], F32, tag="vf", bufs=2)
                    pv = pvf[:, 0:C]
                    nc.tensor.matmul(
                        pv, xnF_v[:, 0:2, t * P:(t + 1) * P], wvp_v,
                        start=True, stop=False, perf_mode=DR)
                    nc.tensor.matmul(
                        pv, xnF_v[:, 2, t * P:(t + 1) * P], wvt,
                        start=False, stop=(not has_bv))
                    if has_bv:
                        nc.tensor.matmul(pv, ones1, bv_sb, start=False,
                                         stop=True)
                    _copy(t, vaug[:, t, :, 0:D],
                          pv.rearrange("p (h d) -> p h d", h=H))

            def emit_S1a(st_, t):
                qF, kF = st_["qF"], st_["kF"]
                # attention: S^T computed as full [128,128] blocks per
                # (head-parity, chunk): both sequences of the pair at once.
                # Cross-sequence quadrants are garbage that the block-diagonal
                # causal mask zeroes before AV. Two psum banks by head parity
                # (HW forbids mixed PE row-groups per bank partition range).
                # em columns: block (hp, ch) at (hp*KC + ch) * 128.
                em = wk.tile([P, H * P], BF16, tag="em", bufs=5)
                for hp in range(2):
                    sthf = ps.tile([P, 512], F32, tag="st", bufs=2,
                                   name="sth")
                    sth = sthf[:, 0:KC * P]
                    pb = hp * 64
                    for ch in range(KC):
                        nc.tensor.matmul(
                            sth[:, ch * P:(ch + 1) * P],
                            kF[ch][pb:pb + 64, t * P:(t + 1) * P],
                            qF[ch][pb:pb + 64, t * P:(t + 1) * P],
                            start=True, stop=True)
                    nc.scalar.activation(
                        out=em[:, hp * KC * P:(hp + 1) * KC * P], in_=sth,
                        func=ACTF.Exp, bias=0.0, scale=SCALE)
                nc.gpsimd.tensor_mul(out=em, in0=em, in1=maskt)
                st_[f"em{t}"] = em

            def emit_S1b(st_, t):
                vaug, attn = st_["vaug"], st_["attn"]
                em = st_.pop(f"em{t}")
                avf = ps.tile([P, 512], F32, tag="avpr", bufs=2)
                av = avf[:, 0:H * (D + 1)].rearrange("p (h e) -> p h e",
                                                     e=D + 1)
                for ch in range(KC):
                    for hp in range(2):
                        h = 2 * ch + hp
                        bc = (hp * KC + ch) * P
                        nc.tensor.matmul(
                            av[:, h, :], em[:, bc:bc + P],
                            vaug[:, t, h, :], start=True, stop=True)
                invl = wk.tile([P, H], F32, tag="invl")
                nc.vector.reciprocal(
                    out=invl, in_=av[:, :, D:D + 1].rearrange("p h 1 -> p h"))
                nc.vector.tensor_mul(
                    out=attn[:, t * C:(t + 1) * C].rearrange(
                        "p (h d) -> p h d", h=H),
                    in0=av[:, :, 0:D],
                    in1=invl.unsqueeze(2).broadcast_to([P, H, D]))

            def emit_D_tile(st_, t):
                i0, attn, x2 = st_["i0"], st_["attn"], st_["x2"]
                # proj + residual + LN2 -> xn2F
                tp = ps.tile([P, C], BF16, tag="big", bufs=2)
                for k in range(KC):
                    nc.tensor.transpose(
                        tp[:, k * P:(k + 1) * P],
                        attn[:, t * C + k * P: t * C + (k + 1) * P], ident)
                aof = wk.tile([P, KC * P], FP8, tag="aoF")
                _copy(t, aof, tp)
                aof_v = aof.rearrange("p (k c) -> p k c", k=KC)
                pprf = ps.tile([P, 512], F32, tag="avpr", bufs=2)
                ppr = pprf[:, 0:C]
                nc.tensor.matmul(ppr, aof_v[:, 0:2, :], wop_v,
                                 start=True, stop=False, perf_mode=DR)
                nc.tensor.matmul(ppr, aof_v[:, 2, :], wot,
                                 start=False, stop=(not has_bo))
                if has_bo:
                    nc.tensor.matmul(ppr, ones1, bo_sb, start=False, stop=True)
                xt = wk.tile([P, C], F32, tag="xd")
                nc.sync.dma_start(
                    out=xt, in_=x_d[(i0 + t) * P:(i0 + t + 1) * P, :])
                nc.vector.tensor_add(out=x2[:, t, :], in0=xt, in1=ppr)
                st_[f"xn0d_{t}"] = ln_chain(x2[:, t, :], "lnxn0D")

            def emit_D3(st_, t):
                tp_to_F(t, st_.pop(f"xn0d_{t}"), st_["xn2F"], eng_off=1)

            def emit_EF_chunk(st_, ph):
                # 8 phases per segment: per group g: E half-j, E half-j,
                # F tiles 0-1, F tiles 2-3
                g = ph // 4
                sub = ph % 4
                if sub == 0:
                    emit_E(st_, g, 0, JC // 2)
                elif sub == 1:
                    emit_E(st_, g, JC // 2, JC)
                elif sub == 2:
                    emit_F(st_, g, 0, 2)
                else:
                    emit_F(st_, g, 2, 4)

            def emit_E(st_, g, j0, j1):
                xn2F = st_["xn2F"]
                xn2F_v = xn2F.rearrange("p (k w) -> p k w", k=KC)
                if f"hF{g}" not in st_:
                    st_[f"hF{g}"] = wk.tile([P, JC, 512], FP8, tag=f"hFg{g}",
                                            bufs=2, name=f"hFg{g}")
                hFg = st_[f"hF{g}"]
                for j in range(j0, j1):
                    phf = ps.tile([P, 512], F32, tag="big", bufs=2)
                    nc.tensor.matmul(
                        phf, w1p_v[:, :, j * P:(j + 1) * P],
                        xn2F_v[:, 0:2, g * 512:(g + 1) * 512],
                        start=True, stop=False, perf_mode=DR)
                    nc.tensor.matmul(
                        phf, w1t[:, j * P:(j + 1) * P],
                        xn2F_v[:, 2, g * 512:(g + 1) * 512],
                        start=False, stop=True)
                    nc.scalar.activation(out=hFg[:, j, :], in_=phf,
                                         func=ACTF.Relu,
                                         bias=bh_sb[:, j:j + 1], scale=1.0)

            def emit_F(st_, g, tg0, tg1):
                i0, x2 = st_["i0"], st_["x2"]
                hFg = st_[f"hF{g}"]
                for tg in range(tg0, tg1):
                    t = g * (512 // P) + tg
                    pff = ps.tile([P, 512], F32, tag="vf", bufs=2)
                    pf = pff[:, 0:C]
                    for p_ in range(JC // 2):
                        nc.tensor.matmul(
                            pf,
                            hFg[:, 2 * p_:2 * p_ + 2, tg * P:(tg + 1) * P],
                            w28_v[:, 2 * p_:2 * p_ + 2, :],
                            start=(p_ == 0),
                            stop=(p_ == JC // 2 - 1 and not has_b2),
                            perf_mode=DR)
                    if has_b2:
                        nc.tensor.matmul(pf, ones1, b2_sb, start=False,
                                         stop=True)
                    ot = wk.tile([P, C], F32, tag="ot")
                    nc.vector.tensor_add(out=ot, in0=x2[:, t, :], in1=pf)
                    nc.sync.dma_start(
                        out=out_d[(i0 + t) * P:(i0 + t + 1) * P, :], in_=ot)

            # ====== software-pipelined emission over segments ======
            # While segment s runs attention/proj (latency-bound, PE-sparse),
            # the instruction streams also carry segment s+1's LN1 loads and
            # segment s-1's FFN groups (PE-dense) to keep every engine fed.
            def _emit_all():
                cur = new_state(0)
                for t in range(SEG):
                    emit_A_ln(cur, t)
                for t in range(SEG):
                    emit_A_tp(cur, t)
                prv = None
                for s in range(nseg):
                    emit_B(cur)
                    nxt = new_state(s + 1) if s + 1 < nseg else None
                    for t in range(SEG + 5):
                        if t < SEG:
                            emit_S1a(cur, t)     # S^T matmuls + exp
                        if 2 <= t <= SEG + 1:
                            emit_S1b(cur, t - 2)  # mask, AV, normalize
                        if 3 <= t <= SEG + 2:
                            emit_D_tile(cur, t - 3)  # proj + resid + LN2 stats
                        if 5 <= t:
                            emit_D3(cur, t - 5)  # LN2 transposes -> xn2F
                        if nxt is not None:
                            if t < SEG:
                                emit_A_ln(nxt, t)
                            if 2 <= t <= SEG + 1:
                                emit_A_tp(nxt, t - 2)
                        if prv is not None and t < 8:
                            emit_EF_chunk(prv, t)
                    prv, cur = cur, nxt
                for ph in range(8):
                    emit_EF_chunk(prv, ph)

            import contextlib
            loop_ctx = tc.For_i(0, loop_n) if loop_n > 1 \
                else contextlib.nullcontext()
            with loop_ctx:
                _emit_all()

    nc.compile()
    return nc


def _bf16(a):
    return np.asarray(a, np.float32).astype(ml_dtypes.bfloat16)


def _fp8(a):
    return np.clip(np.asarray(a, np.float32), -240.0, 240.0).astype(
        ml_dtypes.float8_e4m3)


def _pair_tail(w):
    """[C, M] fp32 -> DoubleRow pair [P, 2*M] (k-tiles 0,1) + tail [P, M]."""
    m = w.shape[1]
    pair = w[0:2 * P].reshape(2, P, m).transpose(1, 0, 2).reshape(P, 2 * m)
    return _fp8(pair), _fp8(w[2 * P:3 * P])


def _prep(ln1_g, ln1_b, Wq, Wk, Wv, Wo, bo, ln2_g, ln2_b, W1, b1, W2, b2):
    """Host-side weight prep: fold LN affine into weights, pack aux consts."""
    ln1_g = np.asarray(ln1_g, np.float32)
    ln1_b = np.asarray(ln1_b, np.float32)
    ln2_g = np.asarray(ln2_g, np.float32)
    ln2_b = np.asarray(ln2_b, np.float32)
    wq_all = np.asarray(Wq, np.float32).transpose(1, 0, 2).reshape(C, C)
    wk_all = np.asarray(Wk, np.float32).transpose(1, 0, 2).reshape(C, C)
    wv_all = np.asarray(Wv, np.float32).transpose(1, 0, 2).reshape(C, C)
    W1 = np.asarray(W1, np.float32)
    W2f = np.asarray(W2, np.float32)
    bq = ln1_b @ wq_all
    bk = ln1_b @ wk_all
    bv = ln1_b @ wv_all
    bh = np.asarray(b1, np.float32) + ln2_b @ W1
    causal_t = np.tril(np.ones((T, T), np.float32)).T  # [s, t]
    mask_bd = np.zeros((P, P), np.float32)  # block-diag causal^T for seq pair
    mask_bd[:T, :T] = causal_t
    mask_bd[T:, T:] = causal_t
    wqp, wqt = _pair_tail(ln1_g[:, None] * wq_all)
    wkp, wkt = _pair_tail(ln1_g[:, None] * wk_all)
    wvp, wvt = _pair_tail(ln1_g[:, None] * wv_all)
    wop, wot = _pair_tail(np.asarray(Wo, np.float32))
    w1p, w1t = _pair_tail(ln2_g[:, None] * W1)
    w28 = _fp8(W2f.reshape(JC, P, C).transpose(1, 0, 2).reshape(P, JC * C))
    d = {
        "wqp": wqp, "wqt": wqt,
        "wkp": wkp, "wkt": wkt,
        "wvp": wvp, "wvt": wvt,
        "wop": wop, "wot": wot,
        "w1p": w1p, "w1t": w1t,
        "w2": w28,
        "bq": bq.reshape(KC, P).T.copy(),
        "bk": bk.reshape(KC, P).T.copy(),
        "bh": bh.reshape(JC, P).T.copy(),
        "bv": _bf16(bv).reshape(1, C),
        "bo_r": _bf16(bo).reshape(1, C),
        "b2_r": _bf16(b2).reshape(1, C),
        "ident": np.eye(P, dtype=np.float32).astype(ml_dtypes.bfloat16),
        "maskt": _bf16(np.tile(mask_bd, (1, H))),
    }
    flags = (bool(np.any(bv != 0)), bool(np.any(np.asarray(bo) != 0)),
             bool(np.any(np.asarray(b2) != 0)))
    return d, flags


def kernel(x, ln1_g, ln1_b, Wq, Wk, Wv, Wo, bo, ln2_g, ln2_b, W1, b1, W2, b2):
    global last_exec_time_ns
    x = np.asarray(x, np.float32)
    aux, flags = _prep(ln1_g, ln1_b, Wq, Wk, Wv, Wo, bo, ln2_g, ln2_b, W1, b1,
                       W2, b2)
    key = flags
    if key not in _CACHE:
        _CACHE[key] = _build(*flags)
    nc = _CACHE[key]
    in_maps = []
    for c in range(N_CORES):
        m = dict(aux)
        m["x"] = x[c * B_LOC:(c + 1) * B_LOC].reshape(NTOK, C)
        in_maps.append(m)
    trace = bool(os.environ.get("BASS_TRACE"))
    try:
        res = run_bass_kernel_spmd(nc, in_maps, list(range(N_CORES)),
                                   trace=trace)
    except ModuleNotFoundError:
        res = run_bass_kernel_spmd(nc, in_maps, list(range(N_CORES)))
    last_exec_time_ns = res.exec_time_ns
    out = np.stack([res.results[c]["out"] for c in range(N_CORES)])
    return out.reshape(B_FULL, T, C).astype(np.float32)
